# revision 32
# baseline (speedup 1.0000x reference)
"""Multi-head self-attention (RoPE, causal) Trainium2 Bass kernel.

Sharding: 8 cores, each handles 2 of 16 heads for both batch elements
(tensor-parallel over heads). Each core computes its heads' QKV projection,
RoPE, causal attention, and a partial output projection over its slice of
the contraction dim; the host sums the 8 partial outputs.

Layouts (per core):
  xt      [2048, 4096]  x^T            (d_model, b*t)        fp32r
  wqkt    [2048, 512]   W_{q,k}^T cols (q0 q1 k0 k1), q/k rows
                        deinterleaved (even dims then odd dims) for RoPE
  wvt     [2048, 256]   W_v^T cols (v0 v1), natural order
  woutt   [256, 2048]   W_out slice^T  (dc, d_model)
  angc/angs [128, 2048] range-reduced RoPE angles (cos/sin path); the ACT
                        Sin LUT is only accurate near [-pi, pi]

All matmuls run as float32r (~1 cycle/row at free dim >= 256, vs 4 for plain
fp32; ~1.6e-4 relative precision). Attention is computed in S^T form:
S^T[j,i] tiles via matmul(lhsT=kT, rhs=qT), exp on ACT (scale=1/sqrt(dk)
folded in), causal masking as a multiplicative DVE mask on diagonal blocks
only (fully-masked blocks are skipped), O^T accumulated via
matmul(lhsT=v_tile, rhs=P^T), row sums via a ones-vector matmul. Softmax
normalization repacks the row sums [1,512] -> [128,4] through a DRAM bounce
so the DVE reciprocal (cost ~ free size) is cheap, then broadcasts the
reciprocals back across partitions with a K=1 ones matmul and scales O^T
in place. RoPE uses host-deinterleaved q/k rows (even dims then odd dims)
so the pair swap is a half-swap permutation matmul on the PE.
"""

import numpy as np
import concourse.bass as bass
import concourse.mybir as mybir
from concourse.bass_utils import run_bass_kernel_spmd
from concourse.tile import TileContext
from concourse.vector_clock import ScopedClock, VectorClock

F32 = mybir.dt.float32
F32R = mybir.dt.float32r
AF = mybir.ActivationFunctionType

D_MODEL = 2048
N_HEADS = 16
DK = 128
B = 2
T = 2048
BT = B * T
N_CORES = 8
HPC = N_HEADS // N_CORES  # heads per core = 2
THETA = 10000.0
SCALE = 1.0 / float(np.sqrt(DK))

CHUNK = 256          # bt columns per QKV chunk
NCHUNK = T // CHUNK  # chunks per batch element
ND = D_MODEL // 128  # d_model tiles = 16
IC = 512             # attention i-chunk width
NIC = T // IC        # i-chunks per batch = 4


class SplitDrainTileContext(TileContext):
    """This walrus build supports at most ONE sync wait per instruction.
    Split multi-wait instructions into single-wait nop chains, and emit the
    end-of-kernel drain as per-proc single-wait drains."""

    def _commit_and_lower(self, inst, original_block, old_bb_map, bb_to_exit_bb):
        si = inst.sync_info
        if si is not None and len(si.on_wait) > 1:
            waits = list(si.on_wait)
            for w in waits[:-1]:
                nop = mybir.InstNoOp(name=self.nc.get_next_instruction_name())
                nop.engine = inst.engine
                nop.sync_info = mybir.SyncInfo(on_wait=[w], on_update=[])
                super()._commit_and_lower(nop, original_block, old_bb_map, bb_to_exit_bb)
            inst.sync_info = mybir.SyncInfo(
                on_wait=[waits[-1]], on_update=list(si.on_update)
            )
        super()._commit_and_lower(inst, original_block, old_bb_map, bb_to_exit_bb)

    def _drain_and_barrier(self, tick_clock, wait_clock):
        gc = tick_clock.global_clock
        nprocs = len(gc)
        for p in range(nprocs):
            t = gc[p]
            if t <= 0:
                continue
            vc = VectorClock([0] * nprocs)
            vc.require_at_least(p, t)
            d = self.nc.sync.drain()
            wait_clock.add_sem_waits(d.ins, ScopedClock({None: vc}))
        self.nc.all_engine_barrier()
        assert self.sems is not None
        popped = self.nc._tile_sem_poison_stack.pop()
        assert popped is self._sem_poison
        self.nc.clear_and_free_semaphores(list(self.sems.allocated().values()))
        self.nc.all_engine_barrier()


def _consts():
    n = np.arange(64, dtype=np.float64)
    inv_freq = THETA ** (-2.0 * n / DK)  # [64]
    invf128 = np.concatenate([inv_freq, inv_freq]).astype(np.float32)[:, None]
    sign128 = np.concatenate([-np.ones(64), np.ones(64)]).astype(np.float32)[:, None]
    perm = np.zeros((128, 128), dtype=np.float32)
    perm[np.arange(64), np.arange(64) + 64] = 1.0
    perm[np.arange(64) + 64, np.arange(64)] = 1.0
    ones128 = np.ones((128, 1), dtype=np.float32)
    onesrow = np.ones((1, 128), dtype=np.float32)
    # single shifted mask: M[j, c] = 1 if j <= c - 384; mask for diagonal
    # offset k is the window M[:, 384-128k : 896-128k]  (= j + 128k <= i)
    j = np.arange(128)[:, None]
    c = np.arange(896)[None, :]
    masks = (j <= c - 384).astype(np.float32)
    return invf128, sign128, perm, ones128, onesrow, masks


def build(debug=False):
    nc = bass.Bass()
    xt_d = nc.declare_dram_parameter("xt", [D_MODEL, BT], F32R, isOutput=False)
    wqkt_d = nc.declare_dram_parameter("wqkt", [D_MODEL, 4 * DK], F32R, isOutput=False)
    wvt_d = nc.declare_dram_parameter("wvt", [D_MODEL, 2 * DK], F32R, isOutput=False)
    wot_d = nc.declare_dram_parameter("woutt", [2 * DK, D_MODEL], F32R, isOutput=False)
    angc_d = nc.declare_dram_parameter("angc", [128, T], F32, isOutput=False)
    angs_d = nc.declare_dram_parameter("angs", [128, T], F32, isOutput=False)
    out_d = nc.declare_dram_parameter("out", [BT, D_MODEL], F32, isOutput=True)
    if debug:
        dbg_cs1 = nc.declare_dram_parameter("dbg_cs1", [128, T], F32, isOutput=True)
        dbg_cs2 = nc.declare_dram_parameter("dbg_cs2", [128, T], F32, isOutput=True)
        dbg_qk = nc.declare_dram_parameter("dbg_qk", [128, 4 * T], F32, isOutput=True)
        dbg_v = nc.declare_dram_parameter("dbg_v", [128, ND * 256], F32, isOutput=True)
        dbg_ot = nc.declare_dram_parameter("dbg_ot", [128, HPC * T], F32, isOutput=True)

    invf128, sign128, perm, ones128, onesrow, masks = _consts()
    sign_c = nc.inline_tensor(sign128, name="sign_c")
    perm_c = nc.inline_tensor(perm, name="perm_c")
    ones128_c = nc.inline_tensor(ones128, name="ones128_c")
    onesrow_c = nc.inline_tensor(onesrow, name="onesrow_c")
    masks_c = nc.inline_tensor(masks, name="masks_c")

    with SplitDrainTileContext(nc) as tc:
        with (
            tc.tile_pool(name="persist", bufs=1) as P,
            tc.tile_pool(name="ps_small", bufs=2, space="PSUM") as PSs,
            tc.tile_pool(name="ps_st", bufs=2, space="PSUM") as PSst,
            tc.tile_pool(name="ps_out", bufs=2, space="PSUM") as PSo,
            tc.tile_pool(name="ps_acc", bufs=1, space="PSUM") as PSacc,
            tc.tile_pool(name="xtp", bufs=2) as XT,
        ):
            # ---- persistent SBUF ----
            wqk = P.tile([128, ND * 512], F32R, tag="wqk")
            wv = P.tile([128, ND * 256], F32R, tag="wv")
            wo = P.tile([128, HPC * D_MODEL], F32R, tag="wo")
            cs1 = P.tile([128, T], F32, tag="cs1")
            cs2 = P.tile([128, T], F32, tag="cs2")
            mask_sb = P.tile([128, 896], F32, tag="mask")
            perm_sb = P.tile([128, 128], F32R, tag="perm")
            ones128_sb = P.tile([128, 1], F32R, tag="ones128")
            onesrow_sb = P.tile([1, 128], F32R, tag="onesrow")
            qk_rot = P.tile([128, 4 * T], F32R, tag="qk_rot")
            v_nat = P.tile([128, ND * 256], F32R, tag="v_nat")
            ot0 = P.tile([128, T], F32R, tag="ot0")
            ot1 = P.tile([128, T], F32R, tag="ot1")
            ot_h = [ot0, ot1]

            # ---- setup ----
            with nc.named_scope("setup"), tc.tile_pool(name="scratch", bufs=1) as S:
                xt0_t = XT.tile([128, ND * CHUNK], F32R, tag="xt")
                nc.sync.dma_start(
                    out=xt0_t[:].rearrange("p (k n) -> p k n", k=ND),
                    in_=xt_d[:, 0:CHUNK].rearrange("(k p) n -> p k n", p=128),
                )
                nc.scalar.dma_start(
                    out=wqk[:, : 8 * 512].rearrange("p (k n) -> p k n", k=8),
                    in_=wqkt_d[: 8 * 128, :].rearrange("(k p) n -> p k n", p=128),
                )
                nc.scalar.dma_start(
                    out=wqk[:, 8 * 512 :].rearrange("p (k n) -> p k n", k=8),
                    in_=wqkt_d[8 * 128 :, :].rearrange("(k p) n -> p k n", p=128),
                )
                nc.scalar.dma_start(
                    out=wv[:].rearrange("p (k n) -> p k n", k=ND),
                    in_=xt_like_rearr(wvt_d, 2 * DK),
                )
                nc.sync.dma_start(out=perm_sb[:], in_=perm_c[:, :].bitcast(F32R))
                nc.sync.dma_start(out=onesrow_sb[:], in_=onesrow_c[:, :].bitcast(F32R))
                nc.sync.dma_start(out=ones128_sb[:], in_=ones128_c[:, :].bitcast(F32R))

                angc_t = S.tile([128, T], F32, tag="angc")
                angs_t = S.tile([128, T], F32, tag="angs")
                sinf = S.tile([128, T], F32, tag="sinf")
                sign_sb = S.tile([128, 1], F32, tag="sign")
                negpi = S.tile([128, 1], F32, tag="negpi")
                nc.scalar.dma_start(out=angc_t[:], in_=angc_d[:, :])
                nc.scalar.dma_start(out=angs_t[:], in_=angs_d[:, :])
                nc.sync.dma_start(out=sign_sb[:], in_=sign_c[:, :])
                nc.gpsimd.memset(negpi[:], -float(np.pi))
                # host supplies angc = mod(ang + 3pi/2, 2pi), angs = mod(ang + pi, 2pi)
                # cos(ang) = sin(angc - pi); sin(ang) = sin(angs - pi)
                nc.scalar.activation(cs1[:], angc_t[:], AF.Sin, bias=negpi[:, 0:1])
                nc.scalar.activation(sinf[:], angs_t[:], AF.Sin, bias=negpi[:, 0:1])
                nc.vector.tensor_scalar_mul(cs2[:], sinf[:], sign_sb[:, 0:1])
                if debug:
                    nc.sync.dma_start(out=dbg_cs1[:, :], in_=cs1[:])
                    nc.sync.dma_start(out=dbg_cs2[:, :], in_=cs2[:])

            with tc.tile_pool(
                name="work", bufs=2
            ) as WK, tc.tile_pool(name="ptp", bufs=2) as PTP, tc.tile_pool(
                name="stage", bufs=2
            ) as STG, tc.tile_pool(name="smallp", bufs=2) as SMP, tc.tile_pool(
                name="rcp", bufs=1
            ) as RCP, tc.tile_pool(name="tbp", bufs=1) as TBP, tc.tile_pool(
                name="rsd", bufs=1, space="DRAM"
            ) as RSD:
                for b in range(B):
                    # ================= QKV + RoPE =================
                    with nc.named_scope(f"qkv_b{b}"):
                        for m in range(NCHUNK):
                            c0 = b * T + m * CHUNK
                            t0 = m * CHUNK
                            if b == 0 and m == 0:
                                xt_t = xt0_t
                            else:
                                xt_t = XT.tile([128, ND * CHUNK], F32R, tag="xt")
                                nc.sync.dma_start(
                                    out=xt_t[:].rearrange("p (k n) -> p k n", k=ND),
                                    in_=xt_d[:, c0 : c0 + CHUNK].rearrange(
                                        "(k p) n -> p k n", p=128
                                    ),
                                )
                            # q0 q1 k0 k1 projections + rope
                            for j in range(4):
                                ps = PSs.tile([128, CHUNK], F32, tag="small")
                                for k in range(ND):
                                    nc.tensor.matmul(
                                        ps[:],
                                        wqk[:, k * 512 + j * 128 : k * 512 + j * 128 + 128],
                                        xt_t[:, k * CHUNK : (k + 1) * CHUNK],
                                        start=(k == 0),
                                        stop=(k == ND - 1),
                                    )
                                raw = WK.tile([128, CHUNK], F32R, tag="raw")
                                nc.scalar.copy(raw[:], ps[:])
                                ps2 = PSs.tile([128, CHUNK], F32, tag="small")
                                nc.tensor.matmul(
                                    ps2[:], perm_sb[:], raw[:], start=True, stop=True
                                )
                                dst = qk_rot[:, j * T + t0 : j * T + t0 + CHUNK]
                                tB = TBP.tile([128, CHUNK], F32, tag="tB")
                                nc.vector.tensor_mul(
                                    dst, raw[:].bitcast(F32), cs1[:, t0 : t0 + CHUNK]
                                )
                                nc.vector.tensor_mul(
                                    tB[:], ps2[:], cs2[:, t0 : t0 + CHUNK]
                                )
                                nc.vector.tensor_add(dst, dst.bitcast(F32), tB[:])
                            # v (natural layout)
                            for tt in range(CHUNK // 128):
                                psv = PSs.tile([128, 2 * DK], F32, tag="small")
                                for k in range(ND):
                                    nc.tensor.matmul(
                                        psv[:],
                                        xt_t[
                                            :,
                                            k * CHUNK + tt * 128 : k * CHUNK + tt * 128 + 128,
                                        ],
                                        wv[:, k * 256 : (k + 1) * 256],
                                        start=(k == 0),
                                        stop=(k == ND - 1),
                                    )
                                jt = m * (CHUNK // 128) + tt
                                nc.scalar.copy(
                                    v_nat[:, jt * 256 : (jt + 1) * 256], psv[:]
                                )

                    if debug and b == 0:
                        nc.sync.dma_start(out=dbg_qk[:, :], in_=qk_rot[:].bitcast(F32))
                        nc.sync.dma_start(out=dbg_v[:, :], in_=v_nat[:].bitcast(F32))

                    if b == 0:
                        nc.sync.dma_start(out=mask_sb[:], in_=masks_c[:, :])

                    # ================= attention =================
                    with nc.named_scope(f"attn_b{b}"):
                        for h in range(HPC):
                            qcol = h * T
                            kcol = (2 + h) * T
                            for ci in range(NIC):
                                i0 = ci * IC
                                njt = 4 * ci + 4
                                ps_ot = PSacc.tile([128, IC], F32, tag="ot")
                                ps_sum = PSacc.tile([1, IC], F32, tag="sum")
                                for jt in range(njt):
                                    j0 = jt * 128
                                    ps_st = PSst.tile([128, IC], F32, tag="st")
                                    nc.tensor.matmul(
                                        ps_st[:],
                                        qk_rot[:, kcol + j0 : kcol + j0 + 128],
                                        qk_rot[:, qcol + i0 : qcol + i0 + IC],
                                        start=True,
                                        stop=True,
                                    )
                                    pt = PTP.tile([128, IC], F32R, tag="pt")
                                    nc.scalar.activation(
                                        pt[:], ps_st[:], AF.Exp, scale=SCALE
                                    )
                                    dk_off = jt - 4 * ci
                                    if dk_off >= 0:
                                        m0 = 384 - 128 * dk_off
                                        nc.vector.tensor_mul(
                                            pt[:],
                                            pt[:].bitcast(F32),
                                            mask_sb[:, m0 : m0 + 512],
                                        )
                                    nc.tensor.matmul(
                                        ps_ot[:],
                                        v_nat[:, jt * 256 + h * 128 : jt * 256 + h * 128 + 128],
                                        pt[:],
                                        start=(jt == 0),
                                        stop=(jt == njt - 1),
                                    )
                                    nc.tensor.matmul(
                                        ps_sum[:],
                                        ones128_sb[:],
                                        pt[:],
                                        start=(jt == 0),
                                        stop=(jt == njt - 1),
                                    )
                                # stash unnormalized O^T; reciprocal of the
                                # row sums via a DRAM repack to [128, 4] (DVE
                                # reciprocal time scales with free size)
                                rsc = SMP.tile([1, IC], F32R, tag="rsc")
                                nc.scalar.copy(rsc[:], ps_sum[:])
                                nc.scalar.copy(
                                    ot_h[h][:, i0 : i0 + IC], ps_ot[:]
                                )
                                rd = RSD.tile([1, IC], F32R, tag="rd")
                                nc.sync.dma_start(out=rd[:, :], in_=rsc[:])
                                rp = RCP.tile([128, IC // 128], F32R, tag="rp")
                                nc.sync.dma_start(
                                    out=rp[:],
                                    in_=rd[:, :].rearrange("a (p c) -> (a p) c", p=128),
                                )
                                rp2 = RCP.tile([128, IC // 128], F32R, tag="rp2")
                                with nc.allow_low_precision("recip of fp32 bits"):
                                    nc.vector.reciprocal(rp2[:], rp[:].bitcast(F32))
                                rd2 = RSD.tile([1, IC], F32R, tag="rd2")
                                nc.sync.dma_start(
                                    out=rd2[:, :].rearrange("a (p c) -> (a p) c", p=128),
                                    in_=rp2[:],
                                )
                                rsc2 = SMP.tile([1, IC], F32R, tag="rsc2")
                                nc.sync.dma_start(out=rsc2[0:1, :], in_=rd2[:, :])
                                ps_bc = PSo.tile([128, IC], F32, tag="out")
                                nc.tensor.matmul(
                                    ps_bc[:], onesrow_sb[:], rsc2[0:1, :],
                                    start=True, stop=True,
                                )
                                dst = ot_h[h][:, i0 : i0 + IC]
                                nc.vector.tensor_mul(dst, dst.bitcast(F32), ps_bc[:])

                    if b == 0:
                        nc.sync.dma_start(
                            out=wo[:].rearrange("p (k n) -> p k n", k=HPC),
                            in_=wot_d[:, :].rearrange("(k p) n -> p k n", p=128),
                        )

                    # ================= output projection =================
                    with nc.named_scope(f"oproj_b{b}"):
                        for n in range(T // 128):
                            for og in range(2):
                                stg = STG.tile([128, 1024], F32, tag="stage")
                                for sc in range(2):
                                    oc = og * 2 + sc
                                    ps_o = PSo.tile([128, 512], F32, tag="out")
                                    for h in range(HPC):
                                        nc.tensor.matmul(
                                            ps_o[:],
                                            ot_h[h][:, n * 128 : n * 128 + 128],
                                            wo[:, h * D_MODEL + oc * 512 : h * D_MODEL + oc * 512 + 512],
                                            start=(h == 0),
                                            stop=(h == HPC - 1),
                                        )
                                    if (n * 2 + oc) % 2 == 0:
                                        nc.scalar.copy(stg[:, sc * 512 : sc * 512 + 512], ps_o[:])
                                    else:
                                        nc.vector.tensor_copy(stg[:, sc * 512 : sc * 512 + 512], ps_o[:])
                                eng = nc.sync if (n * 2 + og) % 2 == 0 else nc.scalar
                                eng.dma_start(
                                    out=out_d[
                                        b * T + n * 128 : b * T + n * 128 + 128,
                                        og * 1024 : og * 1024 + 1024,
                                    ],
                                    in_=stg[:],
                                )
    return nc


def xt_like_rearr(d, ncols):
    return d[:, :].rearrange("(k p) n -> p k n", p=128)


_NC_CACHE = None


def _get_nc():
    global _NC_CACHE
    if _NC_CACHE is None:
        _NC_CACHE = build()
    return _NC_CACHE


def make_in_maps(x, token_positions, W_qkv, W_out):
    x = np.asarray(x, dtype=np.float32)
    W_qkv = np.asarray(W_qkv, dtype=np.float32)
    W_out = np.asarray(W_out, dtype=np.float32)
    posf = np.asarray(token_positions).astype(np.float64)
    n = np.arange(64, dtype=np.float64)
    inv_freq = THETA ** (-2.0 * n / DK)
    invf128 = np.concatenate([inv_freq, inv_freq])
    ang = posf[None, :] * invf128[:, None]  # [128, T]
    angc = np.mod(ang + 3 * np.pi / 2, 2 * np.pi).astype(np.float32)
    angs = np.mod(ang + np.pi, 2 * np.pi).astype(np.float32)

    xt = np.ascontiguousarray(x.reshape(BT, D_MODEL).T)  # [d, bt]

    deint = np.concatenate([np.arange(0, 128, 2), np.arange(1, 128, 2)])  # [128]
    in_maps = []
    for c in range(N_CORES):
        h0 = HPC * c
        qk_rows = []
        for h in (h0, h0 + 1):
            qk_rows.append(h * DK + deint)  # q rows, deinterleaved
        for h in (h0, h0 + 1):
            qk_rows.append(D_MODEL + h * DK + deint)  # k rows
        qk_rows = np.concatenate(qk_rows)
        v_rows = np.concatenate(
            [2 * D_MODEL + h * DK + np.arange(DK) for h in (h0, h0 + 1)]
        )
        o_cols = np.concatenate([h * DK + np.arange(DK) for h in (h0, h0 + 1)])
        in_maps.append(
            {
                "xt": xt,
                "wqkt": np.ascontiguousarray(W_qkv[qk_rows, :].T),
                "wvt": np.ascontiguousarray(W_qkv[v_rows, :].T),
                "woutt": np.ascontiguousarray(W_out[:, o_cols].T),
                "angc": angc,
                "angs": angs,
            }
        )
    return in_maps


def run(inputs, trace=False):
    nc = _get_nc()
    in_maps = make_in_maps(**inputs)
    res = run_bass_kernel_spmd(nc, in_maps, list(range(N_CORES)), trace=trace)
    acc = np.zeros((BT, D_MODEL), dtype=np.float64)
    for r in res.results:
        acc += r["out"]
    out = acc.astype(np.float32).reshape(B, T, D_MODEL)
    return out, res


def kernel(**inputs):
    out, _ = run(inputs, trace=False)
    return out


# revision 34
# speedup vs baseline: 1.0523x; 1.0523x over previous
"""Multi-head self-attention (RoPE, causal) Trainium2 Bass kernel.

Sharding: 8 cores, each handles 2 of 16 heads for both batch elements
(tensor-parallel over heads). Each core computes its heads' QKV projection,
RoPE, causal attention, and a partial output projection over its slice of
the contraction dim; the host sums the 8 partial outputs.

Layouts (per core):
  xt      [2048, 4096]  x^T            (d_model, b*t)        fp32r
  wqkt    [2048, 512]   W_{q,k}^T cols (q0 q1 k0 k1), q/k rows
                        deinterleaved (even dims then odd dims) for RoPE
  wvt     [2048, 256]   W_v^T cols (v0 v1), natural order
  woutt   [256, 2048]   W_out slice^T  (dc, d_model)
  angc/angs [128, 2048] range-reduced RoPE angles (cos/sin path); the ACT
                        Sin LUT is only accurate near [-pi, pi]

All matmuls run as float32r (~1 cycle/row at free dim >= 256, vs 4 for plain
fp32; ~1.6e-4 relative precision). Attention is computed in S^T form:
S^T[j,i] tiles via matmul(lhsT=kT, rhs=qT), exp on ACT (scale=1/sqrt(dk)
folded in), causal masking as a multiplicative DVE mask on diagonal blocks
only (fully-masked blocks are skipped), O^T accumulated via
matmul(lhsT=v_tile, rhs=P^T), row sums via a ones-vector matmul. Softmax
normalization repacks the row sums [1,512] -> [128,4] through a DRAM bounce
so the DVE reciprocal (cost ~ free size) is cheap, then broadcasts the
reciprocals back across partitions with a K=1 ones matmul and scales O^T
in place. RoPE uses host-deinterleaved q/k rows (even dims then odd dims)
so the pair swap is a half-swap permutation matmul on the PE.
"""

import numpy as np
import concourse.bass as bass
import concourse.mybir as mybir
from concourse.bass_utils import run_bass_kernel_spmd
from concourse.tile import TileContext
from concourse.vector_clock import ScopedClock, VectorClock

F32 = mybir.dt.float32
F32R = mybir.dt.float32r
AF = mybir.ActivationFunctionType

D_MODEL = 2048
N_HEADS = 16
DK = 128
B = 2
T = 2048
BT = B * T
N_CORES = 8
HPC = N_HEADS // N_CORES  # heads per core = 2
THETA = 10000.0
SCALE = 1.0 / float(np.sqrt(DK))

CHUNK = 256          # bt columns per QKV chunk
NCHUNK = T // CHUNK  # chunks per batch element
ND = D_MODEL // 128  # d_model tiles = 16
IC = 512             # attention i-chunk width
NIC = T // IC        # i-chunks per batch = 4


class SplitDrainTileContext(TileContext):
    """This walrus build supports at most ONE sync wait per instruction.
    Split multi-wait instructions into single-wait nop chains, and emit the
    end-of-kernel drain as per-proc single-wait drains."""

    def _commit_and_lower(self, inst, original_block, old_bb_map, bb_to_exit_bb):
        si = inst.sync_info
        if si is not None and len(si.on_wait) > 1:
            waits = list(si.on_wait)
            for w in waits[:-1]:
                nop = mybir.InstNoOp(name=self.nc.get_next_instruction_name())
                nop.engine = inst.engine
                nop.sync_info = mybir.SyncInfo(on_wait=[w], on_update=[])
                super()._commit_and_lower(nop, original_block, old_bb_map, bb_to_exit_bb)
            inst.sync_info = mybir.SyncInfo(
                on_wait=[waits[-1]], on_update=list(si.on_update)
            )
        super()._commit_and_lower(inst, original_block, old_bb_map, bb_to_exit_bb)

    def _drain_and_barrier(self, tick_clock, wait_clock):
        gc = tick_clock.global_clock
        nprocs = len(gc)
        for p in range(nprocs):
            t = gc[p]
            if t <= 0:
                continue
            vc = VectorClock([0] * nprocs)
            vc.require_at_least(p, t)
            d = self.nc.sync.drain()
            wait_clock.add_sem_waits(d.ins, ScopedClock({None: vc}))
        self.nc.all_engine_barrier()
        assert self.sems is not None
        popped = self.nc._tile_sem_poison_stack.pop()
        assert popped is self._sem_poison
        self.nc.clear_and_free_semaphores(list(self.sems.allocated().values()))
        self.nc.all_engine_barrier()


def _consts():
    n = np.arange(64, dtype=np.float64)
    inv_freq = THETA ** (-2.0 * n / DK)  # [64]
    invf128 = np.concatenate([inv_freq, inv_freq]).astype(np.float32)[:, None]
    sign128 = np.concatenate([-np.ones(64), np.ones(64)]).astype(np.float32)[:, None]
    perm = np.zeros((128, 128), dtype=np.float32)
    perm[np.arange(64), np.arange(64) + 64] = 1.0
    perm[np.arange(64) + 64, np.arange(64)] = 1.0
    ones128 = np.ones((128, 1), dtype=np.float32)
    onesrow = np.ones((1, 128), dtype=np.float32)
    # single shifted mask: M[j, c] = 1 if j <= c - 384; mask for diagonal
    # offset k is the window M[:, 384-128k : 896-128k]  (= j + 128k <= i)
    j = np.arange(128)[:, None]
    c = np.arange(896)[None, :]
    masks = (j <= c - 384).astype(np.float32)
    return invf128, sign128, perm, ones128, onesrow, masks


def build(debug=False):
    nc = bass.Bass()
    xt_d = nc.declare_dram_parameter("xt", [D_MODEL, BT], F32R, isOutput=False)
    wqkt_d = nc.declare_dram_parameter("wqkt", [D_MODEL, 4 * DK], F32R, isOutput=False)
    wvt_d = nc.declare_dram_parameter("wvt", [D_MODEL, 2 * DK], F32R, isOutput=False)
    wot_d = nc.declare_dram_parameter("woutt", [2 * DK, D_MODEL], F32R, isOutput=False)
    angc_d = nc.declare_dram_parameter("angc", [128, T], F32, isOutput=False)
    angs_d = nc.declare_dram_parameter("angs", [128, T], F32, isOutput=False)
    out_d = nc.declare_dram_parameter("out", [BT, D_MODEL], F32, isOutput=True)
    if debug:
        dbg_cs1 = nc.declare_dram_parameter("dbg_cs1", [128, T], F32, isOutput=True)
        dbg_cs2 = nc.declare_dram_parameter("dbg_cs2", [128, T], F32, isOutput=True)
        dbg_qk = nc.declare_dram_parameter("dbg_qk", [128, 4 * T], F32, isOutput=True)
        dbg_v = nc.declare_dram_parameter("dbg_v", [128, ND * 256], F32, isOutput=True)
        dbg_ot = nc.declare_dram_parameter("dbg_ot", [128, HPC * T], F32, isOutput=True)

    invf128, sign128, perm, ones128, onesrow, masks = _consts()
    sign_c = nc.inline_tensor(sign128, name="sign_c")
    perm_c = nc.inline_tensor(perm, name="perm_c")
    ones128_c = nc.inline_tensor(ones128, name="ones128_c")
    onesrow_c = nc.inline_tensor(onesrow, name="onesrow_c")
    masks_c = nc.inline_tensor(masks, name="masks_c")

    with SplitDrainTileContext(nc) as tc:
        with (
            tc.tile_pool(name="persist", bufs=1) as P,
            tc.tile_pool(name="ps_small", bufs=2, space="PSUM") as PSs,
            tc.tile_pool(name="ps_st", bufs=2, space="PSUM") as PSst,
            tc.tile_pool(name="ps_out", bufs=2, space="PSUM") as PSo,
            tc.tile_pool(name="ps_acc", bufs=1, space="PSUM") as PSacc,
            tc.tile_pool(name="xtp", bufs=2) as XT,
        ):
            # ---- persistent SBUF ----
            wqk = P.tile([128, ND * 512], F32R, tag="wqk")
            wv = P.tile([128, ND * 256], F32R, tag="wv")
            wo = P.tile([128, HPC * D_MODEL], F32R, tag="wo")
            cs1 = P.tile([128, T], F32, tag="cs1")
            cs2 = P.tile([128, T], F32, tag="cs2")
            mask_sb = P.tile([128, 896], F32, tag="mask")
            perm_sb = P.tile([128, 128], F32R, tag="perm")
            ones128_sb = P.tile([128, 1], F32R, tag="ones128")
            onesrow_sb = P.tile([1, 128], F32R, tag="onesrow")
            qk_rot = P.tile([128, 4 * T], F32R, tag="qk_rot")
            v_nat = P.tile([128, ND * 256], F32R, tag="v_nat")
            ot0 = P.tile([128, T], F32R, tag="ot0")
            ot1 = P.tile([128, T], F32R, tag="ot1")
            ot_h = [ot0, ot1]

            # ---- setup ----
            with nc.named_scope("setup"), tc.tile_pool(name="scratch", bufs=1) as S:
                nc.sync.dma_start(out=perm_sb[:], in_=perm_c[:, :].bitcast(F32R))
                xt0_t = XT.tile([128, ND * CHUNK], F32R, tag="xt")
                nc.sync.dma_start(
                    out=xt0_t[:].rearrange("p (k n) -> p k n", k=ND),
                    in_=xt_d[:, 0:CHUNK].rearrange("(k p) n -> p k n", p=128),
                )
                # keep the PE busy (and HAM warm) while weights stream in
                ps_wu = PSs.tile([128, 128], F32, tag="small")
                for r in range(48):
                    nc.tensor.matmul(
                        ps_wu[:], perm_sb[:], perm_sb[:],
                        start=(r == 0), stop=(r == 47),
                    )
                nc.sync.dma_start(
                    out=wqk[:, : 8 * 512].rearrange("p (k n) -> p k n", k=8),
                    in_=wqkt_d[: 8 * 128, :].rearrange("(k p) n -> p k n", p=128),
                )
                nc.sync.dma_start(
                    out=wqk[:, 8 * 512 :].rearrange("p (k n) -> p k n", k=8),
                    in_=wqkt_d[8 * 128 :, :].rearrange("(k p) n -> p k n", p=128),
                )
                nc.sync.dma_start(
                    out=wv[:].rearrange("p (k n) -> p k n", k=ND),
                    in_=xt_like_rearr(wvt_d, 2 * DK),
                )
                nc.sync.dma_start(out=onesrow_sb[:], in_=onesrow_c[:, :].bitcast(F32R))
                nc.sync.dma_start(out=ones128_sb[:], in_=ones128_c[:, :].bitcast(F32R))

                angc_t = S.tile([128, T], F32, tag="angc")
                angs_t = S.tile([128, T], F32, tag="angs")
                sinf = S.tile([128, T], F32, tag="sinf")
                sign_sb = S.tile([128, 1], F32, tag="sign")
                negpi = S.tile([128, 1], F32, tag="negpi")
                nc.sync.dma_start(out=angc_t[:], in_=angc_d[:, :])
                nc.sync.dma_start(out=angs_t[:], in_=angs_d[:, :])
                nc.sync.dma_start(out=sign_sb[:], in_=sign_c[:, :])
                nc.gpsimd.memset(negpi[:], -float(np.pi))
                # host supplies angc = mod(ang + 3pi/2, 2pi), angs = mod(ang + pi, 2pi)
                # cos(ang) = sin(angc - pi); sin(ang) = sin(angs - pi)
                nc.scalar.activation(cs1[:], angc_t[:], AF.Sin, bias=negpi[:, 0:1])
                nc.scalar.activation(sinf[:], angs_t[:], AF.Sin, bias=negpi[:, 0:1])
                nc.vector.tensor_scalar_mul(cs2[:], sinf[:], sign_sb[:, 0:1])
                if debug:
                    nc.sync.dma_start(out=dbg_cs1[:, :], in_=cs1[:])
                    nc.sync.dma_start(out=dbg_cs2[:, :], in_=cs2[:])

            with tc.tile_pool(
                name="work", bufs=2
            ) as WK, tc.tile_pool(name="ptp", bufs=2) as PTP, tc.tile_pool(
                name="stage", bufs=2
            ) as STG, tc.tile_pool(name="smallp", bufs=2) as SMP, tc.tile_pool(
                name="rcp", bufs=1
            ) as RCP, tc.tile_pool(name="tbp", bufs=1) as TBP, tc.tile_pool(
                name="rsd", bufs=1, space="DRAM"
            ) as RSD:
                for b in range(B):
                    # ================= QKV + RoPE =================
                    with nc.named_scope(f"qkv_b{b}"):
                        for m in range(NCHUNK):
                            c0 = b * T + m * CHUNK
                            t0 = m * CHUNK
                            if b == 0 and m == 0:
                                xt_t = xt0_t
                            else:
                                xt_t = XT.tile([128, ND * CHUNK], F32R, tag="xt")
                                nc.sync.dma_start(
                                    out=xt_t[:].rearrange("p (k n) -> p k n", k=ND),
                                    in_=xt_d[:, c0 : c0 + CHUNK].rearrange(
                                        "(k p) n -> p k n", p=128
                                    ),
                                )
                            # q0 q1 k0 k1 projections + rope
                            for j in range(4):
                                ps = PSs.tile([128, CHUNK], F32, tag="small")
                                for k in range(ND):
                                    nc.tensor.matmul(
                                        ps[:],
                                        wqk[:, k * 512 + j * 128 : k * 512 + j * 128 + 128],
                                        xt_t[:, k * CHUNK : (k + 1) * CHUNK],
                                        start=(k == 0),
                                        stop=(k == ND - 1),
                                    )
                                raw = WK.tile([128, CHUNK], F32R, tag="raw")
                                nc.scalar.copy(raw[:], ps[:])
                                ps2 = PSs.tile([128, CHUNK], F32, tag="small")
                                nc.tensor.matmul(
                                    ps2[:], perm_sb[:], raw[:], start=True, stop=True
                                )
                                dst = qk_rot[:, j * T + t0 : j * T + t0 + CHUNK]
                                tB = TBP.tile([128, CHUNK], F32, tag="tB")
                                nc.vector.tensor_mul(
                                    dst, raw[:].bitcast(F32), cs1[:, t0 : t0 + CHUNK]
                                )
                                nc.vector.tensor_mul(
                                    tB[:], ps2[:], cs2[:, t0 : t0 + CHUNK]
                                )
                                nc.vector.tensor_add(dst, dst.bitcast(F32), tB[:])
                            # v (natural layout)
                            for tt in range(CHUNK // 128):
                                psv = PSs.tile([128, 2 * DK], F32, tag="small")
                                for k in range(ND):
                                    nc.tensor.matmul(
                                        psv[:],
                                        xt_t[
                                            :,
                                            k * CHUNK + tt * 128 : k * CHUNK + tt * 128 + 128,
                                        ],
                                        wv[:, k * 256 : (k + 1) * 256],
                                        start=(k == 0),
                                        stop=(k == ND - 1),
                                    )
                                jt = m * (CHUNK // 128) + tt
                                nc.scalar.copy(
                                    v_nat[:, jt * 256 : (jt + 1) * 256], psv[:]
                                )

                    if debug and b == 0:
                        nc.sync.dma_start(out=dbg_qk[:, :], in_=qk_rot[:].bitcast(F32))
                        nc.sync.dma_start(out=dbg_v[:, :], in_=v_nat[:].bitcast(F32))

                    if b == 0:
                        nc.sync.dma_start(out=mask_sb[:], in_=masks_c[:, :])

                    # ================= attention =================
                    with nc.named_scope(f"attn_b{b}"):
                        for h in range(HPC):
                            qcol = h * T
                            kcol = (2 + h) * T
                            for ci in range(NIC):
                                i0 = ci * IC
                                njt = 4 * ci + 4
                                ps_ot = PSacc.tile([128, IC], F32, tag="ot")
                                ps_sum = PSacc.tile([1, IC], F32, tag="sum")
                                for jt in range(njt):
                                    j0 = jt * 128
                                    ps_st = PSst.tile([128, IC], F32, tag="st")
                                    nc.tensor.matmul(
                                        ps_st[:],
                                        qk_rot[:, kcol + j0 : kcol + j0 + 128],
                                        qk_rot[:, qcol + i0 : qcol + i0 + IC],
                                        start=True,
                                        stop=True,
                                    )
                                    pt = PTP.tile([128, IC], F32R, tag="pt")
                                    nc.scalar.activation(
                                        pt[:], ps_st[:], AF.Exp, scale=SCALE
                                    )
                                    dk_off = jt - 4 * ci
                                    if dk_off >= 0:
                                        m0 = 384 - 128 * dk_off
                                        nc.vector.tensor_mul(
                                            pt[:],
                                            pt[:].bitcast(F32),
                                            mask_sb[:, m0 : m0 + 512],
                                        )
                                    nc.tensor.matmul(
                                        ps_ot[:],
                                        v_nat[:, jt * 256 + h * 128 : jt * 256 + h * 128 + 128],
                                        pt[:],
                                        start=(jt == 0),
                                        stop=(jt == njt - 1),
                                    )
                                    nc.tensor.matmul(
                                        ps_sum[:],
                                        ones128_sb[:],
                                        pt[:],
                                        start=(jt == 0),
                                        stop=(jt == njt - 1),
                                    )
                                # stash unnormalized O^T; reciprocal of the
                                # row sums via a DRAM repack to [128, 4] (DVE
                                # reciprocal time scales with free size)
                                rsc = SMP.tile([1, IC], F32R, tag="rsc")
                                nc.scalar.copy(rsc[:], ps_sum[:])
                                nc.scalar.copy(
                                    ot_h[h][:, i0 : i0 + IC], ps_ot[:]
                                )
                                rd = RSD.tile([1, IC], F32R, tag="rd")
                                nc.sync.dma_start(out=rd[:, :], in_=rsc[:])
                                rp = RCP.tile([128, IC // 128], F32R, tag="rp")
                                nc.sync.dma_start(
                                    out=rp[:],
                                    in_=rd[:, :].rearrange("a (p c) -> (a p) c", p=128),
                                )
                                rp2 = RCP.tile([128, IC // 128], F32R, tag="rp2")
                                with nc.allow_low_precision("recip of fp32 bits"):
                                    nc.vector.reciprocal(rp2[:], rp[:].bitcast(F32))
                                rd2 = RSD.tile([1, IC], F32R, tag="rd2")
                                nc.sync.dma_start(
                                    out=rd2[:, :].rearrange("a (p c) -> (a p) c", p=128),
                                    in_=rp2[:],
                                )
                                rsc2 = SMP.tile([1, IC], F32R, tag="rsc2")
                                nc.sync.dma_start(out=rsc2[0:1, :], in_=rd2[:, :])
                                ps_bc = PSo.tile([128, IC], F32, tag="out")
                                nc.tensor.matmul(
                                    ps_bc[:], onesrow_sb[:], rsc2[0:1, :],
                                    start=True, stop=True,
                                )
                                dst = ot_h[h][:, i0 : i0 + IC]
                                nc.vector.tensor_mul(dst, dst.bitcast(F32), ps_bc[:])

                    if b == 0:
                        nc.sync.dma_start(
                            out=wo[:].rearrange("p (k n) -> p k n", k=HPC),
                            in_=wot_d[:, :].rearrange("(k p) n -> p k n", p=128),
                        )

                    # ================= output projection =================
                    with nc.named_scope(f"oproj_b{b}"):
                        for n in range(T // 128):
                            for og in range(2):
                                stg = STG.tile([128, 1024], F32, tag="stage")
                                for sc in range(2):
                                    oc = og * 2 + sc
                                    ps_o = PSo.tile([128, 512], F32, tag="out")
                                    for h in range(HPC):
                                        nc.tensor.matmul(
                                            ps_o[:],
                                            ot_h[h][:, n * 128 : n * 128 + 128],
                                            wo[:, h * D_MODEL + oc * 512 : h * D_MODEL + oc * 512 + 512],
                                            start=(h == 0),
                                            stop=(h == HPC - 1),
                                        )
                                    if (n * 2 + oc) % 2 == 0:
                                        nc.scalar.copy(stg[:, sc * 512 : sc * 512 + 512], ps_o[:])
                                    else:
                                        nc.vector.tensor_copy(stg[:, sc * 512 : sc * 512 + 512], ps_o[:])
                                nc.sync.dma_start(
                                    out=out_d[
                                        b * T + n * 128 : b * T + n * 128 + 128,
                                        og * 1024 : og * 1024 + 1024,
                                    ],
                                    in_=stg[:],
                                )
    return nc


def xt_like_rearr(d, ncols):
    return d[:, :].rearrange("(k p) n -> p k n", p=128)


_NC_CACHE = None


def _get_nc():
    global _NC_CACHE
    if _NC_CACHE is None:
        _NC_CACHE = build()
    return _NC_CACHE


def make_in_maps(x, token_positions, W_qkv, W_out):
    x = np.asarray(x, dtype=np.float32)
    W_qkv = np.asarray(W_qkv, dtype=np.float32)
    W_out = np.asarray(W_out, dtype=np.float32)
    posf = np.asarray(token_positions).astype(np.float64)
    n = np.arange(64, dtype=np.float64)
    inv_freq = THETA ** (-2.0 * n / DK)
    invf128 = np.concatenate([inv_freq, inv_freq])
    ang = posf[None, :] * invf128[:, None]  # [128, T]
    angc = np.mod(ang + 3 * np.pi / 2, 2 * np.pi).astype(np.float32)
    angs = np.mod(ang + np.pi, 2 * np.pi).astype(np.float32)

    xt = np.ascontiguousarray(x.reshape(BT, D_MODEL).T)  # [d, bt]

    deint = np.concatenate([np.arange(0, 128, 2), np.arange(1, 128, 2)])  # [128]
    in_maps = []
    for c in range(N_CORES):
        h0 = HPC * c
        qk_rows = []
        for h in (h0, h0 + 1):
            qk_rows.append(h * DK + deint)  # q rows, deinterleaved
        for h in (h0, h0 + 1):
            qk_rows.append(D_MODEL + h * DK + deint)  # k rows
        qk_rows = np.concatenate(qk_rows)
        v_rows = np.concatenate(
            [2 * D_MODEL + h * DK + np.arange(DK) for h in (h0, h0 + 1)]
        )
        o_cols = np.concatenate([h * DK + np.arange(DK) for h in (h0, h0 + 1)])
        in_maps.append(
            {
                "xt": xt,
                "wqkt": np.ascontiguousarray(W_qkv[qk_rows, :].T),
                "wvt": np.ascontiguousarray(W_qkv[v_rows, :].T),
                "woutt": np.ascontiguousarray(W_out[:, o_cols].T),
                "angc": angc,
                "angs": angs,
            }
        )
    return in_maps


def run(inputs, trace=False):
    nc = _get_nc()
    in_maps = make_in_maps(**inputs)
    res = run_bass_kernel_spmd(nc, in_maps, list(range(N_CORES)), trace=trace)
    acc = np.zeros((BT, D_MODEL), dtype=np.float64)
    for r in res.results:
        acc += r["out"]
    out = acc.astype(np.float32).reshape(B, T, D_MODEL)
    return out, res


def kernel(**inputs):
    out, _ = run(inputs, trace=False)
    return out


# revision 35
# speedup vs baseline: 1.0565x; 1.0040x over previous
"""Multi-head self-attention (RoPE, causal) Trainium2 Bass kernel.

Sharding: 8 cores, each handles 2 of 16 heads for both batch elements
(tensor-parallel over heads). Each core computes its heads' QKV projection,
RoPE, causal attention, and a partial output projection over its slice of
the contraction dim; the host sums the 8 partial outputs.

Layouts (per core):
  xt      [2048, 4096]  x^T            (d_model, b*t)        fp32r
  wqkt    [2048, 512]   W_{q,k}^T cols (q0 q1 k0 k1), q/k rows
                        deinterleaved (even dims then odd dims) for RoPE
  wvt     [2048, 256]   W_v^T cols (v0 v1), natural order
  woutt   [256, 2048]   W_out slice^T  (dc, d_model)
  angc/angs [128, 2048] range-reduced RoPE angles (cos/sin path); the ACT
                        Sin LUT is only accurate near [-pi, pi]

All matmuls run as float32r (~1 cycle/row at free dim >= 256, vs 4 for plain
fp32; ~1.6e-4 relative precision). Attention is computed in S^T form:
S^T[j,i] tiles via matmul(lhsT=kT, rhs=qT), exp on ACT (scale=1/sqrt(dk)
folded in), causal masking as a multiplicative DVE mask on diagonal blocks
only (fully-masked blocks are skipped), O^T accumulated via
matmul(lhsT=v_tile, rhs=P^T), row sums via a ones-vector matmul. Softmax
normalization repacks the row sums [1,512] -> [128,4] through a DRAM bounce
so the DVE reciprocal (cost ~ free size) is cheap, then broadcasts the
reciprocals back across partitions with a K=1 ones matmul and scales O^T
in place. RoPE uses host-deinterleaved q/k rows (even dims then odd dims)
so the pair swap is a half-swap permutation matmul on the PE.
"""

import numpy as np
import concourse.bass as bass
import concourse.mybir as mybir
from concourse.bass_utils import run_bass_kernel_spmd
from concourse.tile import TileContext
from concourse.vector_clock import ScopedClock, VectorClock

F32 = mybir.dt.float32
F32R = mybir.dt.float32r
AF = mybir.ActivationFunctionType

D_MODEL = 2048
N_HEADS = 16
DK = 128
B = 2
T = 2048
BT = B * T
N_CORES = 8
HPC = N_HEADS // N_CORES  # heads per core = 2
THETA = 10000.0
SCALE = 1.0 / float(np.sqrt(DK))

CHUNK = 256          # bt columns per QKV chunk
NCHUNK = T // CHUNK  # chunks per batch element
ND = D_MODEL // 128  # d_model tiles = 16
IC = 512             # attention i-chunk width
NIC = T // IC        # i-chunks per batch = 4


class SplitDrainTileContext(TileContext):
    """This walrus build supports at most ONE sync wait per instruction.
    Split multi-wait instructions into single-wait nop chains, and emit the
    end-of-kernel drain as per-proc single-wait drains."""

    def _commit_and_lower(self, inst, original_block, old_bb_map, bb_to_exit_bb):
        si = inst.sync_info
        if si is not None and len(si.on_wait) > 1:
            waits = list(si.on_wait)
            for w in waits[:-1]:
                nop = mybir.InstNoOp(name=self.nc.get_next_instruction_name())
                nop.engine = inst.engine
                nop.sync_info = mybir.SyncInfo(on_wait=[w], on_update=[])
                super()._commit_and_lower(nop, original_block, old_bb_map, bb_to_exit_bb)
            inst.sync_info = mybir.SyncInfo(
                on_wait=[waits[-1]], on_update=list(si.on_update)
            )
        super()._commit_and_lower(inst, original_block, old_bb_map, bb_to_exit_bb)

    def _drain_and_barrier(self, tick_clock, wait_clock):
        gc = tick_clock.global_clock
        nprocs = len(gc)
        for p in range(nprocs):
            t = gc[p]
            if t <= 0:
                continue
            vc = VectorClock([0] * nprocs)
            vc.require_at_least(p, t)
            d = self.nc.sync.drain()
            wait_clock.add_sem_waits(d.ins, ScopedClock({None: vc}))
        self.nc.all_engine_barrier()
        assert self.sems is not None
        popped = self.nc._tile_sem_poison_stack.pop()
        assert popped is self._sem_poison
        self.nc.clear_and_free_semaphores(list(self.sems.allocated().values()))
        self.nc.all_engine_barrier()


def _consts():
    n = np.arange(64, dtype=np.float64)
    inv_freq = THETA ** (-2.0 * n / DK)  # [64]
    invf128 = np.concatenate([inv_freq, inv_freq]).astype(np.float32)[:, None]
    sign128 = np.concatenate([-np.ones(64), np.ones(64)]).astype(np.float32)[:, None]
    perm = np.zeros((128, 128), dtype=np.float32)
    perm[np.arange(64), np.arange(64) + 64] = 1.0
    perm[np.arange(64) + 64, np.arange(64)] = 1.0
    ones128 = np.ones((128, 1), dtype=np.float32)
    onesrow = np.ones((1, 128), dtype=np.float32)
    # single shifted mask: M[j, c] = 1 if j <= c - 384; mask for diagonal
    # offset k is the window M[:, 384-128k : 896-128k]  (= j + 128k <= i)
    j = np.arange(128)[:, None]
    c = np.arange(896)[None, :]
    masks = (j <= c - 384).astype(np.float32)
    return invf128, sign128, perm, ones128, onesrow, masks


def build(debug=False):
    nc = bass.Bass()
    xt_d = nc.declare_dram_parameter("xt", [D_MODEL, BT], F32R, isOutput=False)
    wqkt_d = nc.declare_dram_parameter("wqkt", [D_MODEL, 4 * DK], F32R, isOutput=False)
    wvt_d = nc.declare_dram_parameter("wvt", [D_MODEL, 2 * DK], F32R, isOutput=False)
    wot_d = nc.declare_dram_parameter("woutt", [2 * DK, D_MODEL], F32R, isOutput=False)
    angc_d = nc.declare_dram_parameter("angc", [128, T], F32, isOutput=False)
    angs_d = nc.declare_dram_parameter("angs", [128, T], F32, isOutput=False)
    out_d = nc.declare_dram_parameter("out", [BT, D_MODEL], F32, isOutput=True)
    if debug:
        dbg_cs1 = nc.declare_dram_parameter("dbg_cs1", [128, T], F32, isOutput=True)
        dbg_cs2 = nc.declare_dram_parameter("dbg_cs2", [128, T], F32, isOutput=True)
        dbg_qk = nc.declare_dram_parameter("dbg_qk", [128, 4 * T], F32, isOutput=True)
        dbg_v = nc.declare_dram_parameter("dbg_v", [128, ND * 256], F32, isOutput=True)
        dbg_ot = nc.declare_dram_parameter("dbg_ot", [128, HPC * T], F32, isOutput=True)

    invf128, sign128, perm, ones128, onesrow, masks = _consts()
    sign_c = nc.inline_tensor(sign128, name="sign_c")
    perm_c = nc.inline_tensor(perm, name="perm_c")
    ones128_c = nc.inline_tensor(ones128, name="ones128_c")
    onesrow_c = nc.inline_tensor(onesrow, name="onesrow_c")
    masks_c = nc.inline_tensor(masks, name="masks_c")

    with SplitDrainTileContext(nc) as tc:
        with (
            tc.tile_pool(name="persist", bufs=1) as P,
            tc.tile_pool(name="ps_small", bufs=2, space="PSUM") as PSs,
            tc.tile_pool(name="ps_st", bufs=2, space="PSUM") as PSst,
            tc.tile_pool(name="ps_out", bufs=2, space="PSUM") as PSo,
            tc.tile_pool(name="ps_acc", bufs=1, space="PSUM") as PSacc,
            tc.tile_pool(name="xtp", bufs=2) as XT,
        ):
            # ---- persistent SBUF ----
            wqk = P.tile([128, ND * 512], F32R, tag="wqk")
            wv = P.tile([128, ND * 256], F32R, tag="wv")
            wo = P.tile([128, HPC * D_MODEL], F32R, tag="wo")
            cs1 = P.tile([128, T], F32, tag="cs1")
            cs2 = P.tile([128, T], F32, tag="cs2")
            mask_sb = P.tile([128, 896], F32, tag="mask")
            perm_sb = P.tile([128, 128], F32R, tag="perm")
            ones128_sb = P.tile([128, 1], F32R, tag="ones128")
            onesrow_sb = P.tile([1, 128], F32R, tag="onesrow")
            qk_rot = P.tile([128, 4 * T], F32R, tag="qk_rot")
            v_nat = P.tile([128, ND * 256], F32R, tag="v_nat")
            ot0 = P.tile([128, T], F32R, tag="ot0")
            ot1 = P.tile([128, T], F32R, tag="ot1")
            ot_h = [ot0, ot1]

            # ---- setup ----
            with nc.named_scope("setup"), tc.tile_pool(name="scratch", bufs=1) as S:
                nc.sync.dma_start(out=perm_sb[:], in_=perm_c[:, :].bitcast(F32R))
                xt0_t = XT.tile([128, ND * CHUNK], F32R, tag="xt")
                nc.sync.dma_start(
                    out=xt0_t[:].rearrange("p (k n) -> p k n", k=ND),
                    in_=xt_d[:, 0:CHUNK].rearrange("(k p) n -> p k n", p=128),
                )
                # keep the PE busy (and HAM warm) while weights stream in
                ps_wu = PSs.tile([128, 128], F32, tag="small")
                for r in range(130):
                    nc.tensor.matmul(
                        ps_wu[:], perm_sb[:], perm_sb[:],
                        start=(r == 0), stop=(r == 129),
                    )
                nc.sync.dma_start(
                    out=wqk[:, : 8 * 512].rearrange("p (k n) -> p k n", k=8),
                    in_=wqkt_d[: 8 * 128, :].rearrange("(k p) n -> p k n", p=128),
                )
                nc.sync.dma_start(
                    out=wqk[:, 8 * 512 :].rearrange("p (k n) -> p k n", k=8),
                    in_=wqkt_d[8 * 128 :, :].rearrange("(k p) n -> p k n", p=128),
                )
                nc.sync.dma_start(
                    out=wv[:].rearrange("p (k n) -> p k n", k=ND),
                    in_=xt_like_rearr(wvt_d, 2 * DK),
                )
                nc.sync.dma_start(out=onesrow_sb[:], in_=onesrow_c[:, :].bitcast(F32R))
                nc.sync.dma_start(out=ones128_sb[:], in_=ones128_c[:, :].bitcast(F32R))

                angc_t = S.tile([128, T], F32, tag="angc")
                angs_t = S.tile([128, T], F32, tag="angs")
                sinf = S.tile([128, T], F32, tag="sinf")
                sign_sb = S.tile([128, 1], F32, tag="sign")
                negpi = S.tile([128, 1], F32, tag="negpi")
                nc.sync.dma_start(out=angc_t[:], in_=angc_d[:, :])
                nc.sync.dma_start(out=angs_t[:], in_=angs_d[:, :])
                nc.sync.dma_start(out=sign_sb[:], in_=sign_c[:, :])
                nc.gpsimd.memset(negpi[:], -float(np.pi))
                # host supplies angc = mod(ang + 3pi/2, 2pi), angs = mod(ang + pi, 2pi)
                # cos(ang) = sin(angc - pi); sin(ang) = sin(angs - pi)
                nc.scalar.activation(cs1[:], angc_t[:], AF.Sin, bias=negpi[:, 0:1])
                nc.scalar.activation(sinf[:], angs_t[:], AF.Sin, bias=negpi[:, 0:1])
                nc.vector.tensor_scalar_mul(cs2[:], sinf[:], sign_sb[:, 0:1])
                if debug:
                    nc.sync.dma_start(out=dbg_cs1[:, :], in_=cs1[:])
                    nc.sync.dma_start(out=dbg_cs2[:, :], in_=cs2[:])

            with tc.tile_pool(
                name="work", bufs=2
            ) as WK, tc.tile_pool(name="ptp", bufs=2) as PTP, tc.tile_pool(
                name="stage", bufs=2
            ) as STG, tc.tile_pool(name="smallp", bufs=2) as SMP, tc.tile_pool(
                name="rcp", bufs=1
            ) as RCP, tc.tile_pool(name="tbp", bufs=1) as TBP, tc.tile_pool(
                name="rsd", bufs=1, space="DRAM"
            ) as RSD:
                for b in range(B):
                    # ================= QKV + RoPE =================
                    with nc.named_scope(f"qkv_b{b}"):
                        for m in range(NCHUNK):
                            c0 = b * T + m * CHUNK
                            t0 = m * CHUNK
                            if b == 0 and m == 0:
                                xt_t = xt0_t
                            else:
                                xt_t = XT.tile([128, ND * CHUNK], F32R, tag="xt")
                                nc.sync.dma_start(
                                    out=xt_t[:].rearrange("p (k n) -> p k n", k=ND),
                                    in_=xt_d[:, c0 : c0 + CHUNK].rearrange(
                                        "(k p) n -> p k n", p=128
                                    ),
                                )
                            # q0 q1 k0 k1 projections + rope
                            for j in range(4):
                                ps = PSs.tile([128, CHUNK], F32, tag="small")
                                for k in range(ND):
                                    nc.tensor.matmul(
                                        ps[:],
                                        wqk[:, k * 512 + j * 128 : k * 512 + j * 128 + 128],
                                        xt_t[:, k * CHUNK : (k + 1) * CHUNK],
                                        start=(k == 0),
                                        stop=(k == ND - 1),
                                    )
                                raw = WK.tile([128, CHUNK], F32R, tag="raw")
                                nc.scalar.copy(raw[:], ps[:])
                                ps2 = PSs.tile([128, CHUNK], F32, tag="small")
                                nc.tensor.matmul(
                                    ps2[:], perm_sb[:], raw[:], start=True, stop=True
                                )
                                dst = qk_rot[:, j * T + t0 : j * T + t0 + CHUNK]
                                tB = TBP.tile([128, CHUNK], F32, tag="tB")
                                nc.vector.tensor_mul(
                                    dst, raw[:].bitcast(F32), cs1[:, t0 : t0 + CHUNK]
                                )
                                nc.vector.tensor_mul(
                                    tB[:], ps2[:], cs2[:, t0 : t0 + CHUNK]
                                )
                                nc.vector.tensor_add(dst, dst.bitcast(F32), tB[:])
                            # v (natural layout)
                            for tt in range(CHUNK // 128):
                                psv = PSs.tile([128, 2 * DK], F32, tag="small")
                                for k in range(ND):
                                    nc.tensor.matmul(
                                        psv[:],
                                        xt_t[
                                            :,
                                            k * CHUNK + tt * 128 : k * CHUNK + tt * 128 + 128,
                                        ],
                                        wv[:, k * 256 : (k + 1) * 256],
                                        start=(k == 0),
                                        stop=(k == ND - 1),
                                    )
                                jt = m * (CHUNK // 128) + tt
                                nc.scalar.copy(
                                    v_nat[:, jt * 256 : (jt + 1) * 256], psv[:]
                                )

                    if debug and b == 0:
                        nc.sync.dma_start(out=dbg_qk[:, :], in_=qk_rot[:].bitcast(F32))
                        nc.sync.dma_start(out=dbg_v[:, :], in_=v_nat[:].bitcast(F32))

                    if b == 0:
                        nc.sync.dma_start(out=mask_sb[:], in_=masks_c[:, :])

                    # ================= attention =================
                    with nc.named_scope(f"attn_b{b}"):
                        for h in range(HPC):
                            qcol = h * T
                            kcol = (2 + h) * T
                            for ci in range(NIC):
                                i0 = ci * IC
                                njt = 4 * ci + 4
                                ps_ot = PSacc.tile([128, IC], F32, tag="ot")
                                ps_sum = PSacc.tile([1, IC], F32, tag="sum")
                                for jt in range(njt):
                                    j0 = jt * 128
                                    ps_st = PSst.tile([128, IC], F32, tag="st")
                                    nc.tensor.matmul(
                                        ps_st[:],
                                        qk_rot[:, kcol + j0 : kcol + j0 + 128],
                                        qk_rot[:, qcol + i0 : qcol + i0 + IC],
                                        start=True,
                                        stop=True,
                                    )
                                    pt = PTP.tile([128, IC], F32R, tag="pt")
                                    nc.scalar.activation(
                                        pt[:], ps_st[:], AF.Exp, scale=SCALE
                                    )
                                    dk_off = jt - 4 * ci
                                    if dk_off >= 0:
                                        m0 = 384 - 128 * dk_off
                                        nc.vector.tensor_mul(
                                            pt[:],
                                            pt[:].bitcast(F32),
                                            mask_sb[:, m0 : m0 + 512],
                                        )
                                    nc.tensor.matmul(
                                        ps_ot[:],
                                        v_nat[:, jt * 256 + h * 128 : jt * 256 + h * 128 + 128],
                                        pt[:],
                                        start=(jt == 0),
                                        stop=(jt == njt - 1),
                                    )
                                    nc.tensor.matmul(
                                        ps_sum[:],
                                        ones128_sb[:],
                                        pt[:],
                                        start=(jt == 0),
                                        stop=(jt == njt - 1),
                                    )
                                # stash unnormalized O^T; reciprocal of the
                                # row sums via a DRAM repack to [128, 4] (DVE
                                # reciprocal time scales with free size)
                                rsc = SMP.tile([1, IC], F32R, tag="rsc")
                                nc.scalar.copy(rsc[:], ps_sum[:])
                                nc.scalar.copy(
                                    ot_h[h][:, i0 : i0 + IC], ps_ot[:]
                                )
                                rd = RSD.tile([1, IC], F32R, tag="rd")
                                nc.sync.dma_start(out=rd[:, :], in_=rsc[:])
                                rp = RCP.tile([128, IC // 128], F32R, tag="rp")
                                nc.sync.dma_start(
                                    out=rp[:],
                                    in_=rd[:, :].rearrange("a (p c) -> (a p) c", p=128),
                                )
                                rp2 = RCP.tile([128, IC // 128], F32R, tag="rp2")
                                with nc.allow_low_precision("recip of fp32 bits"):
                                    nc.vector.reciprocal(rp2[:], rp[:].bitcast(F32))
                                rd2 = RSD.tile([1, IC], F32R, tag="rd2")
                                nc.sync.dma_start(
                                    out=rd2[:, :].rearrange("a (p c) -> (a p) c", p=128),
                                    in_=rp2[:],
                                )
                                rsc2 = SMP.tile([1, IC], F32R, tag="rsc2")
                                nc.sync.dma_start(out=rsc2[0:1, :], in_=rd2[:, :])
                                ps_bc = PSo.tile([128, IC], F32, tag="out")
                                nc.tensor.matmul(
                                    ps_bc[:], onesrow_sb[:], rsc2[0:1, :],
                                    start=True, stop=True,
                                )
                                dst = ot_h[h][:, i0 : i0 + IC]
                                nc.vector.tensor_mul(dst, dst.bitcast(F32), ps_bc[:])

                    if b == 0:
                        nc.sync.dma_start(
                            out=wo[:].rearrange("p (k n) -> p k n", k=HPC),
                            in_=wot_d[:, :].rearrange("(k p) n -> p k n", p=128),
                        )

                    # ================= output projection =================
                    with nc.named_scope(f"oproj_b{b}"):
                        for n in range(T // 128):
                            for og in range(2):
                                stg = STG.tile([128, 1024], F32, tag="stage")
                                for sc in range(2):
                                    oc = og * 2 + sc
                                    ps_o = PSo.tile([128, 512], F32, tag="out")
                                    for h in range(HPC):
                                        nc.tensor.matmul(
                                            ps_o[:],
                                            ot_h[h][:, n * 128 : n * 128 + 128],
                                            wo[:, h * D_MODEL + oc * 512 : h * D_MODEL + oc * 512 + 512],
                                            start=(h == 0),
                                            stop=(h == HPC - 1),
                                        )
                                    if (n * 2 + oc) % 2 == 0:
                                        nc.scalar.copy(stg[:, sc * 512 : sc * 512 + 512], ps_o[:])
                                    else:
                                        nc.vector.tensor_copy(stg[:, sc * 512 : sc * 512 + 512], ps_o[:])
                                nc.sync.dma_start(
                                    out=out_d[
                                        b * T + n * 128 : b * T + n * 128 + 128,
                                        og * 1024 : og * 1024 + 1024,
                                    ],
                                    in_=stg[:],
                                )
    return nc


def xt_like_rearr(d, ncols):
    return d[:, :].rearrange("(k p) n -> p k n", p=128)


_NC_CACHE = None


def _get_nc():
    global _NC_CACHE
    if _NC_CACHE is None:
        _NC_CACHE = build()
    return _NC_CACHE


def make_in_maps(x, token_positions, W_qkv, W_out):
    x = np.asarray(x, dtype=np.float32)
    W_qkv = np.asarray(W_qkv, dtype=np.float32)
    W_out = np.asarray(W_out, dtype=np.float32)
    posf = np.asarray(token_positions).astype(np.float64)
    n = np.arange(64, dtype=np.float64)
    inv_freq = THETA ** (-2.0 * n / DK)
    invf128 = np.concatenate([inv_freq, inv_freq])
    ang = posf[None, :] * invf128[:, None]  # [128, T]
    angc = np.mod(ang + 3 * np.pi / 2, 2 * np.pi).astype(np.float32)
    angs = np.mod(ang + np.pi, 2 * np.pi).astype(np.float32)

    xt = np.ascontiguousarray(x.reshape(BT, D_MODEL).T)  # [d, bt]

    deint = np.concatenate([np.arange(0, 128, 2), np.arange(1, 128, 2)])  # [128]
    in_maps = []
    for c in range(N_CORES):
        h0 = HPC * c
        qk_rows = []
        for h in (h0, h0 + 1):
            qk_rows.append(h * DK + deint)  # q rows, deinterleaved
        for h in (h0, h0 + 1):
            qk_rows.append(D_MODEL + h * DK + deint)  # k rows
        qk_rows = np.concatenate(qk_rows)
        v_rows = np.concatenate(
            [2 * D_MODEL + h * DK + np.arange(DK) for h in (h0, h0 + 1)]
        )
        o_cols = np.concatenate([h * DK + np.arange(DK) for h in (h0, h0 + 1)])
        in_maps.append(
            {
                "xt": xt,
                "wqkt": np.ascontiguousarray(W_qkv[qk_rows, :].T),
                "wvt": np.ascontiguousarray(W_qkv[v_rows, :].T),
                "woutt": np.ascontiguousarray(W_out[:, o_cols].T),
                "angc": angc,
                "angs": angs,
            }
        )
    return in_maps


def run(inputs, trace=False):
    nc = _get_nc()
    in_maps = make_in_maps(**inputs)
    res = run_bass_kernel_spmd(nc, in_maps, list(range(N_CORES)), trace=trace)
    acc = np.zeros((BT, D_MODEL), dtype=np.float64)
    for r in res.results:
        acc += r["out"]
    out = acc.astype(np.float32).reshape(B, T, D_MODEL)
    return out, res


def kernel(**inputs):
    out, _ = run(inputs, trace=False)
    return out
